# revision 14
# baseline (speedup 1.0000x reference)
"""Pointer-network sparse attention on 8 Trainium2 NeuronCores.

scores[p, n] = sum_e relu( (emb[g_p] @ Wx^T)[n, e] + (pred[p] @ Wy^T)[e]
                           + bx[e] + by[e] ) * ws[e]
masked with -inf for n >= graph_sizes[g_p].

Strategy
--------
* The per-pointer projection in the reference re-projects each gathered graph
  (512 projections). Projecting each of the 64 graphs once is 8x fewer FLOPs;
  pointers then reuse their graph's projection X^T.
* Graphs are dealt to the 8 cores round-robin by descending pointer count, so
  one SPMD program (uniform slot capacities = per-rank max) fits every core
  with ~11% padding. A pointer lives on the core that owns its graph, so there
  are no collectives and no data-dependent addressing.
* Per core: PE computes X^T[e, n] per graph (e on partitions) and
  y^T[e, slot] = Wy^T pred + (bx+by).  DVE computes t = relu(X^T + y_col)
  with a fused per-partition add+max.  PE contracts t with ws via an
  accumulating matmul whose stationary is a sliding window over a buffer
  holding ws at one column: pair r of slots lands in PSUM row r, giving a
  single [npairs, 512] result tile instead of 512 one-partition copies.
* Host does the cheap O(MB) reshapes: pre-transposing operands into
  [contraction-on-partition] layout, slot packing, final unpermute + -inf mask.
"""

import numpy as np
import ml_dtypes

N_GRAPHS, MAX_NODES, DIM, N_PTR = 64, 256, 1024, 512
N_CORES = 8
G_PER_CORE = N_GRAPHS // N_CORES  # 8
KT = DIM // 128  # 8 contraction tiles
NEG_INF = np.float32(-np.inf)

# matmul / relu operand dtype ("bfloat16" or "float32")
MM_DTYPE = "bfloat16"
# of every 12 relu tiles, how many go to ScalarE (rest on VectorE)
ACT_RELU_OF_12 = 4


def _np_dt(name):
    return ml_dtypes.bfloat16 if name == "bfloat16" else np.float32


def _build_program(npairs, slot_graph, n_slots):
    """One SPMD program; per-core behaviour differs only through data.

    slot_graph[s] = local graph index (0..7) whose X^T slot s reads.
    n_slots = 2 * npairs (slot count padded to even).
    """
    import concourse.bacc as bacc
    import concourse.tile as tile
    from concourse import mybir
    from concourse.bass import ts, ds

    mdt = getattr(mybir.dt, MM_DTYPE)
    f32 = mybir.dt.float32
    wwin = 2 * npairs - 1  # ws window buffer width

    nc = bacc.Bacc()
    embT_d = nc.declare_dram_parameter("embT", [KT, 128, G_PER_CORE, MAX_NODES], mdt, isOutput=False)
    predT_d = nc.declare_dram_parameter("predT", [KT, 128, n_slots], mdt, isOutput=False)
    WxT_d = nc.declare_dram_parameter("WxT", [KT, 128, DIM], mdt, isOutput=False)
    WyT_d = nc.declare_dram_parameter("WyT", [KT, 128, DIM], mdt, isOutput=False)
    bias_d = nc.declare_dram_parameter("biasv", [128, KT], f32, isOutput=False)
    wsbig_d = nc.declare_dram_parameter("wsbig", [KT, 128, wwin], mdt, isOutput=False)
    out_d = nc.declare_dram_parameter("scores_out", [npairs, 512], f32, isOutput=True)

    with tile.TileContext(nc) as tc:
        with (
            tc.tile_pool(name="const", bufs=1) as cpool,
            tc.tile_pool(name="t", bufs=6) as tpool,
            tc.tile_pool(name="psx", bufs=5, space="PSUM") as psx,
            tc.tile_pool(name="psy", bufs=2, space="PSUM") as psy,
            tc.tile_pool(name="pss", bufs=1, space="PSUM") as pss,
        ):
            embT_sb = cpool.tile([128, KT, G_PER_CORE, MAX_NODES], mdt)
            WxT_sb = cpool.tile([128, KT, DIM], mdt)
            WyT_sb = cpool.tile([128, KT, DIM], mdt)
            predT_sb = cpool.tile([128, KT, n_slots], mdt)
            wsbig_sb = cpool.tile([128, KT, wwin], mdt)
            bias_sb = cpool.tile([128, KT], f32)
            XT_sb = cpool.tile([128, KT, G_PER_CORE, MAX_NODES], mdt)
            yT_sb = cpool.tile([128, KT, n_slots], f32)
            scores_sb = cpool.tile([npairs, 512], f32)

            # emb/Wx lead (X-proj et=0 is the critical path); the small
            # y-proj/score operands slot into the middle of the emb stream.
            nc.sync.dma_start(bias_sb[:], bias_d[:])
            for dt in range(KT // 2):
                nc.sync.dma_start(embT_sb[:, dt], embT_d[dt])
                nc.sync.dma_start(WxT_sb[:, dt], WxT_d[dt])
            for dt in range(KT):
                nc.sync.dma_start(predT_sb[:, dt], predT_d[dt])
                nc.sync.dma_start(wsbig_sb[:, dt], wsbig_d[dt])
                nc.sync.dma_start(WyT_sb[:, dt], WyT_d[dt])
            for dt in range(KT // 2, KT):
                nc.sync.dma_start(embT_sb[:, dt], embT_d[dt])
                nc.sync.dma_start(WxT_sb[:, dt], WxT_d[dt])

            # y^T[e, s] = sum_d Wy[e, d] pred[s, d] + (bx + by)[e]
            for et in range(KT):
                py = psy.tile([128, n_slots], f32)
                for dt in range(KT):
                    nc.tensor.matmul(
                        py[:],
                        WyT_sb[:, dt, ts(et, 128)],
                        predT_sb[:, dt, :],
                        start=(dt == 0),
                        stop=(dt == KT - 1),
                    )
                nc.vector.tensor_scalar(
                    yT_sb[:, et, :], py[:], bias_sb[:, et : et + 1], None,
                    mybir.AluOpType.add,
                )

            # Interleaved per e-tile: X^T[e, n] = sum_d Wx[e, d] emb[n, d]
            # (two graphs per psum tile, dt-outer so 4 consecutive matmuls
            # share lhsT), immediately followed by the score contraction for
            # k-tile kt = et. PSUM accumulation is order-free, so emitting
            # the score chain kt-outer lets PE/DVE/ACT pipeline across
            # e-tiles instead of serializing X-proj before all scores.
            ps = pss.tile([npairs, 512], f32)
            n_mm = npairs * KT
            mm = 0

            def emit_score(kt, p2):
                nonlocal mm
                s0, s1 = 2 * p2, 2 * p2 + 1
                t = tpool.tile([128, 512], mdt, name="t", tag="t")
                for half, s in ((0, s0), (1, s1)):
                    # relu(X^T + y): balance between DVE and ACT
                    if (mm * 2 + half) % 12 < ACT_RELU_OF_12:
                        nc.scalar.activation(
                            t[:, ts(half, 256)],
                            XT_sb[:, kt, slot_graph[s], :],
                            mybir.ActivationFunctionType.Relu,
                            bias=yT_sb[:, kt, s : s + 1],
                            scale=1.0,
                        )
                    else:
                        nc.vector.tensor_scalar(
                            t[:, ts(half, 256)],
                            XT_sb[:, kt, slot_graph[s], :],
                            yT_sb[:, kt, s : s + 1],
                            0.0,
                            mybir.AluOpType.add,
                            mybir.AluOpType.max,
                        )
                nc.tensor.matmul(
                    ps[:],
                    wsbig_sb[:, kt, ds(npairs - 1 - p2, npairs)],
                    t[:],
                    start=(mm == 0),
                    stop=(mm == n_mm - 1),
                )
                mm += 1

            # One-iteration skew: scores for k-tile et-1 are emitted in chunks
            # between the dt-groups of X-proj(et), so PE always has X matmuls
            # queued while DVE/ACT produce relu tiles.
            chunk = (npairs + KT - 1) // KT  # score pairs per dt-group
            for et in range(KT + 1):
                skt = et - 1  # score k-tile trailing the X-proj e-tile
                if et < KT:
                    pxs = [psx.tile([128, 512], f32, name="px", tag="px") for _ in range(G_PER_CORE // 2)]
                    for dt in range(KT):
                        for l2 in range(G_PER_CORE // 2):
                            nc.tensor.matmul(
                                pxs[l2][:],
                                WxT_sb[:, dt, ts(et, 128)],
                                embT_sb[:, dt, 2 * l2 : 2 * l2 + 2, :],
                                start=(dt == 0),
                                stop=(dt == KT - 1),
                            )
                        if skt >= 0:
                            for p2 in range(dt * chunk, min((dt + 1) * chunk, npairs)):
                                emit_score(skt, p2)
                    for l2 in range(G_PER_CORE // 2):
                        nc.scalar.copy(XT_sb[:, et, 2 * l2 : 2 * l2 + 2, :], pxs[l2][:])
                else:
                    for p2 in range(npairs):
                        emit_score(skt, p2)

            nc.vector.tensor_copy(scores_sb[:], ps[:])
            nc.sync.dma_start(out_d[:], scores_sb[:])

    nc.finalize()
    return nc


def _prepare(graph_sizes, embeddings, predicted_embs, graph_ids, Wx, bx, Wy, by, ws):
    np_dt = _np_dt(MM_DTYPE)
    counts = np.bincount(graph_ids, minlength=N_GRAPHS)
    order = np.argsort(-counts, kind="stable")  # rank -> graph id

    S = [int(counts[order[G_PER_CORE * l]]) for l in range(G_PER_CORE)]
    T = sum(S)
    n_slots = T + (T % 2)
    npairs = n_slots // 2

    slot_graph = []
    for l in range(G_PER_CORE):
        slot_graph += [l] * S[l]
    if T % 2:
        slot_graph.append(slot_graph[-1] if slot_graph else 0)

    slot_start = np.cumsum([0] + S)

    WxT = np.ascontiguousarray(Wx.T).reshape(KT, 128, DIM).astype(np_dt)
    WyT = np.ascontiguousarray(Wy.T).reshape(KT, 128, DIM).astype(np_dt)
    biasv = np.ascontiguousarray((bx + by).astype(np.float32).reshape(KT, 128).T)
    wwin = 2 * npairs - 1
    wsbig = np.zeros((KT, 128, wwin), dtype=np_dt)
    wsbig[:, :, npairs - 1] = ws.reshape(KT, 128).astype(np_dt)

    in_maps = []
    ptr_of_slot = []  # per core: global pointer index per slot (-1 = pad)
    for j in range(N_CORES):
        graphs_j = order[[G_PER_CORE * l + j for l in range(G_PER_CORE)]]
        embT = np.ascontiguousarray(
            embeddings[graphs_j].transpose(2, 0, 1)
        ).reshape(KT, 128, G_PER_CORE, MAX_NODES).astype(np_dt)

        slots = np.full(n_slots, -1, dtype=np.int64)
        predc = np.zeros((DIM, n_slots), dtype=np.float32)
        for l in range(G_PER_CORE):
            ptrs = np.nonzero(graph_ids == graphs_j[l])[0]
            k = len(ptrs)
            slots[slot_start[l] : slot_start[l] + k] = ptrs
            predc[:, slot_start[l] : slot_start[l] + k] = predicted_embs[ptrs].T
        ptr_of_slot.append(slots)
        predT = np.ascontiguousarray(predc.reshape(KT, 128, n_slots)).astype(np_dt)

        in_maps.append({
            "embT": embT, "predT": predT, "WxT": WxT, "WyT": WyT,
            "biasv": biasv, "wsbig": wsbig,
        })

    return in_maps, ptr_of_slot, npairs, slot_graph, n_slots


def _run(inputs, trace=False):
    from concourse.bass_utils import run_bass_kernel_spmd

    graph_sizes = np.asarray(inputs["graph_sizes"])
    embeddings = np.asarray(inputs["embeddings"], dtype=np.float32)
    predicted_embs = np.asarray(inputs["predicted_embs"], dtype=np.float32)
    graph_ids = np.asarray(inputs["graph_ids"]).astype(np.int64)
    Wx = np.asarray(inputs["Wx"], dtype=np.float32)
    bx = np.asarray(inputs["bx"], dtype=np.float32)
    Wy = np.asarray(inputs["Wy"], dtype=np.float32)
    by = np.asarray(inputs["by"], dtype=np.float32)
    ws = np.asarray(inputs["ws"], dtype=np.float32)

    in_maps, ptr_of_slot, npairs, slot_graph, n_slots = _prepare(
        graph_sizes, embeddings, predicted_embs, graph_ids, Wx, bx, Wy, by, ws
    )
    nc = _build_program(npairs, slot_graph, n_slots)
    res = run_bass_kernel_spmd(nc, in_maps, list(range(N_CORES)), trace=trace)

    scores = np.zeros((N_PTR, MAX_NODES), dtype=np.float32)
    for j in range(N_CORES):
        rows = np.asarray(res.results[j]["scores_out"], dtype=np.float32)
        rows = rows.reshape(n_slots, MAX_NODES)
        valid = ptr_of_slot[j] >= 0
        scores[ptr_of_slot[j][valid]] = rows[valid]

    sizes = graph_sizes[graph_ids].astype(np.int64)
    pad = np.arange(MAX_NODES, dtype=np.int64)[None, :] >= sizes[:, None]
    scores = np.where(pad, NEG_INF, scores)
    return scores, res


def kernel(**inputs):
    scores, _ = _run(inputs, trace=False)
    return scores


# revision 15
# speedup vs baseline: 1.0640x; 1.0640x over previous
"""Pointer-network sparse attention on 8 Trainium2 NeuronCores.

scores[p, n] = sum_e relu( (emb[g_p] @ Wx^T)[n, e] + (pred[p] @ Wy^T)[e]
                           + bx[e] + by[e] ) * ws[e]
masked with -inf for n >= graph_sizes[g_p].

Strategy
--------
* The per-pointer projection in the reference re-projects each gathered graph
  (512 projections). Projecting each of the 64 graphs once is 8x fewer FLOPs;
  pointers then reuse their graph's projection X^T.
* Graphs are dealt to the 8 cores round-robin by descending pointer count, so
  one SPMD program (uniform slot capacities = per-rank max) fits every core
  with ~11% padding. A pointer lives on the core that owns its graph, so there
  are no collectives and no data-dependent addressing.
* Per core: PE computes X^T[e, n] per graph (e on partitions) and
  y^T[e, slot] = Wy^T pred + (bx+by).  DVE computes t = relu(X^T + y_col)
  with a fused per-partition add+max.  PE contracts t with ws via an
  accumulating matmul whose stationary is a sliding window over a buffer
  holding ws at one column: pair r of slots lands in PSUM row r, giving a
  single [npairs, 512] result tile instead of 512 one-partition copies.
* Host does the cheap O(MB) reshapes: pre-transposing operands into
  [contraction-on-partition] layout, slot packing, final unpermute + -inf mask.
"""

import numpy as np
import ml_dtypes

N_GRAPHS, MAX_NODES, DIM, N_PTR = 64, 256, 1024, 512
N_CORES = 8
G_PER_CORE = N_GRAPHS // N_CORES  # 8
KT = DIM // 128  # 8 contraction tiles
NEG_INF = np.float32(-np.inf)

# matmul / relu operand dtype ("bfloat16" or "float32")
MM_DTYPE = "bfloat16"
# of every 12 relu tiles, how many go to ScalarE (rest on VectorE)
ACT_RELU_OF_12 = 4


def _np_dt(name):
    return ml_dtypes.bfloat16 if name == "bfloat16" else np.float32


def _build_program(npairs, slot_graph, n_slots):
    """One SPMD program; per-core behaviour differs only through data.

    slot_graph[s] = local graph index (0..7) whose X^T slot s reads.
    n_slots = 2 * npairs (slot count padded to even).
    """
    import concourse.bacc as bacc
    import concourse.tile as tile
    from concourse import mybir
    from concourse.bass import ts, ds

    mdt = getattr(mybir.dt, MM_DTYPE)
    f32 = mybir.dt.float32
    wwin = 2 * npairs - 1  # ws window buffer width

    nc = bacc.Bacc()
    embT_d = nc.declare_dram_parameter("embT", [KT, 128, G_PER_CORE, MAX_NODES], mdt, isOutput=False)
    predT_d = nc.declare_dram_parameter("predT", [KT, 128, n_slots], mdt, isOutput=False)
    WxT_d = nc.declare_dram_parameter("WxT", [KT, 128, DIM], mdt, isOutput=False)
    WyT_d = nc.declare_dram_parameter("WyT", [KT, 128, DIM], mdt, isOutput=False)
    bias_d = nc.declare_dram_parameter("biasv", [128, KT], f32, isOutput=False)
    wsbig_d = nc.declare_dram_parameter("wsbig", [KT, 128, wwin], mdt, isOutput=False)
    out_d = nc.declare_dram_parameter("scores_out", [npairs, 512], f32, isOutput=True)

    with tile.TileContext(nc) as tc:
        with (
            tc.tile_pool(name="const", bufs=1) as cpool,
            tc.tile_pool(name="t", bufs=6) as tpool,
            tc.tile_pool(name="psx", bufs=5, space="PSUM") as psx,
            tc.tile_pool(name="psy", bufs=2, space="PSUM") as psy,
            tc.tile_pool(name="pss", bufs=1, space="PSUM") as pss,
        ):
            embT_sb = cpool.tile([128, KT, G_PER_CORE, MAX_NODES], mdt)
            WxT_sb = cpool.tile([128, KT, DIM], mdt)
            WyT_sb = cpool.tile([128, KT, DIM], mdt)
            predT_sb = cpool.tile([128, KT, n_slots], mdt)
            wsbig_sb = cpool.tile([128, KT, wwin], mdt)
            bias_sb = cpool.tile([128, KT], f32)
            XT_sb = cpool.tile([128, KT, G_PER_CORE, MAX_NODES], mdt)
            yT_sb = cpool.tile([128, KT, n_slots], f32)
            scores_sb = cpool.tile([npairs, 512], f32)

            # emb/Wx first: X-proj(et=0) is the critical path at kernel start
            nc.sync.dma_start(bias_sb[:], bias_d[:])
            for dt in range(KT):
                nc.sync.dma_start(embT_sb[:, dt], embT_d[dt])
                nc.sync.dma_start(WxT_sb[:, dt], WxT_d[dt])
            for dt in range(KT):
                nc.sync.dma_start(predT_sb[:, dt], predT_d[dt])
                nc.sync.dma_start(WyT_sb[:, dt], WyT_d[dt])
                nc.sync.dma_start(wsbig_sb[:, dt], wsbig_d[dt])

            # y^T[e, s] = sum_d Wy[e, d] pred[s, d] + (bx + by)[e]
            for et in range(KT):
                py = psy.tile([128, n_slots], f32)
                for dt in range(KT):
                    nc.tensor.matmul(
                        py[:],
                        WyT_sb[:, dt, ts(et, 128)],
                        predT_sb[:, dt, :],
                        start=(dt == 0),
                        stop=(dt == KT - 1),
                    )
                nc.vector.tensor_scalar(
                    yT_sb[:, et, :], py[:], bias_sb[:, et : et + 1], None,
                    mybir.AluOpType.add,
                )

            # Interleaved per e-tile: X^T[e, n] = sum_d Wx[e, d] emb[n, d]
            # (two graphs per psum tile, dt-outer so 4 consecutive matmuls
            # share lhsT), immediately followed by the score contraction for
            # k-tile kt = et. PSUM accumulation is order-free, so emitting
            # the score chain kt-outer lets PE/DVE/ACT pipeline across
            # e-tiles instead of serializing X-proj before all scores.
            ps = pss.tile([npairs, 512], f32)
            n_mm = npairs * KT
            mm = 0

            def emit_score(kt, p2):
                nonlocal mm
                s0, s1 = 2 * p2, 2 * p2 + 1
                t = tpool.tile([128, 512], mdt, name="t", tag="t")
                for half, s in ((0, s0), (1, s1)):
                    # relu(X^T + y): balance between DVE and ACT
                    if (mm * 2 + half) % 12 < ACT_RELU_OF_12:
                        nc.scalar.activation(
                            t[:, ts(half, 256)],
                            XT_sb[:, kt, slot_graph[s], :],
                            mybir.ActivationFunctionType.Relu,
                            bias=yT_sb[:, kt, s : s + 1],
                            scale=1.0,
                        )
                    else:
                        nc.vector.tensor_scalar(
                            t[:, ts(half, 256)],
                            XT_sb[:, kt, slot_graph[s], :],
                            yT_sb[:, kt, s : s + 1],
                            0.0,
                            mybir.AluOpType.add,
                            mybir.AluOpType.max,
                        )
                nc.tensor.matmul(
                    ps[:],
                    wsbig_sb[:, kt, ds(npairs - 1 - p2, npairs)],
                    t[:],
                    start=(mm == 0),
                    stop=(mm == n_mm - 1),
                )
                mm += 1

            # One-iteration skew: scores for k-tile et-1 are emitted in chunks
            # between the dt-groups of X-proj(et), so PE always has X matmuls
            # queued while DVE/ACT produce relu tiles.
            chunk = (npairs + KT - 1) // KT  # score pairs per dt-group
            for et in range(KT + 1):
                skt = et - 1  # score k-tile trailing the X-proj e-tile
                if et < KT:
                    pxs = [psx.tile([128, 512], f32, name="px", tag="px") for _ in range(G_PER_CORE // 2)]
                    for dt in range(KT):
                        for l2 in range(G_PER_CORE // 2):
                            nc.tensor.matmul(
                                pxs[l2][:],
                                WxT_sb[:, dt, ts(et, 128)],
                                embT_sb[:, dt, 2 * l2 : 2 * l2 + 2, :],
                                start=(dt == 0),
                                stop=(dt == KT - 1),
                            )
                        if skt >= 0:
                            for p2 in range(dt * chunk, min((dt + 1) * chunk, npairs)):
                                emit_score(skt, p2)
                    for l2 in range(G_PER_CORE // 2):
                        nc.scalar.copy(XT_sb[:, et, 2 * l2 : 2 * l2 + 2, :], pxs[l2][:])
                else:
                    for p2 in range(npairs):
                        emit_score(skt, p2)

            nc.vector.tensor_copy(scores_sb[:], ps[:])
            nc.sync.dma_start(out_d[:], scores_sb[:])

    nc.finalize()
    return nc


def _prepare(graph_sizes, embeddings, predicted_embs, graph_ids, Wx, bx, Wy, by, ws):
    np_dt = _np_dt(MM_DTYPE)
    counts = np.bincount(graph_ids, minlength=N_GRAPHS)
    order = np.argsort(-counts, kind="stable")  # rank -> graph id

    S = [int(counts[order[G_PER_CORE * l]]) for l in range(G_PER_CORE)]
    T = sum(S)
    n_slots = T + (T % 2)
    npairs = n_slots // 2

    slot_graph = []
    for l in range(G_PER_CORE):
        slot_graph += [l] * S[l]
    if T % 2:
        slot_graph.append(slot_graph[-1] if slot_graph else 0)

    slot_start = np.cumsum([0] + S)

    WxT = np.ascontiguousarray(Wx.T).reshape(KT, 128, DIM).astype(np_dt)
    WyT = np.ascontiguousarray(Wy.T).reshape(KT, 128, DIM).astype(np_dt)
    biasv = np.ascontiguousarray((bx + by).astype(np.float32).reshape(KT, 128).T)
    wwin = 2 * npairs - 1
    wsbig = np.zeros((KT, 128, wwin), dtype=np_dt)
    wsbig[:, :, npairs - 1] = ws.reshape(KT, 128).astype(np_dt)

    in_maps = []
    ptr_of_slot = []  # per core: global pointer index per slot (-1 = pad)
    for j in range(N_CORES):
        graphs_j = order[[G_PER_CORE * l + j for l in range(G_PER_CORE)]]
        embT = np.ascontiguousarray(
            embeddings[graphs_j].transpose(2, 0, 1)
        ).reshape(KT, 128, G_PER_CORE, MAX_NODES).astype(np_dt)

        slots = np.full(n_slots, -1, dtype=np.int64)
        predc = np.zeros((DIM, n_slots), dtype=np.float32)
        for l in range(G_PER_CORE):
            ptrs = np.nonzero(graph_ids == graphs_j[l])[0]
            k = len(ptrs)
            slots[slot_start[l] : slot_start[l] + k] = ptrs
            predc[:, slot_start[l] : slot_start[l] + k] = predicted_embs[ptrs].T
        ptr_of_slot.append(slots)
        predT = np.ascontiguousarray(predc.reshape(KT, 128, n_slots)).astype(np_dt)

        in_maps.append({
            "embT": embT, "predT": predT, "WxT": WxT, "WyT": WyT,
            "biasv": biasv, "wsbig": wsbig,
        })

    return in_maps, ptr_of_slot, npairs, slot_graph, n_slots


def _run(inputs, trace=False):
    from concourse.bass_utils import run_bass_kernel_spmd

    graph_sizes = np.asarray(inputs["graph_sizes"])
    embeddings = np.asarray(inputs["embeddings"], dtype=np.float32)
    predicted_embs = np.asarray(inputs["predicted_embs"], dtype=np.float32)
    graph_ids = np.asarray(inputs["graph_ids"]).astype(np.int64)
    Wx = np.asarray(inputs["Wx"], dtype=np.float32)
    bx = np.asarray(inputs["bx"], dtype=np.float32)
    Wy = np.asarray(inputs["Wy"], dtype=np.float32)
    by = np.asarray(inputs["by"], dtype=np.float32)
    ws = np.asarray(inputs["ws"], dtype=np.float32)

    in_maps, ptr_of_slot, npairs, slot_graph, n_slots = _prepare(
        graph_sizes, embeddings, predicted_embs, graph_ids, Wx, bx, Wy, by, ws
    )
    nc = _build_program(npairs, slot_graph, n_slots)
    res = run_bass_kernel_spmd(nc, in_maps, list(range(N_CORES)), trace=trace)

    scores = np.zeros((N_PTR, MAX_NODES), dtype=np.float32)
    for j in range(N_CORES):
        rows = np.asarray(res.results[j]["scores_out"], dtype=np.float32)
        rows = rows.reshape(n_slots, MAX_NODES)
        valid = ptr_of_slot[j] >= 0
        scores[ptr_of_slot[j][valid]] = rows[valid]

    sizes = graph_sizes[graph_ids].astype(np.int64)
    pad = np.arange(MAX_NODES, dtype=np.int64)[None, :] >= sizes[:, None]
    scores = np.where(pad, NEG_INF, scores)
    return scores, res


def kernel(**inputs):
    scores, _ = _run(inputs, trace=False)
    return scores


# revision 16
# speedup vs baseline: 1.0950x; 1.0291x over previous
"""Pointer-network sparse attention on 8 Trainium2 NeuronCores.

scores[p, n] = sum_e relu( (emb[g_p] @ Wx^T)[n, e] + (pred[p] @ Wy^T)[e]
                           + bx[e] + by[e] ) * ws[e]
masked with -inf for n >= graph_sizes[g_p].

Strategy
--------
* The per-pointer projection in the reference re-projects each gathered graph
  (512 projections). Projecting each of the 64 graphs once is 8x fewer FLOPs;
  pointers then reuse their graph's projection X^T.
* Graphs are dealt to the 8 cores round-robin by descending pointer count, so
  one SPMD program (uniform slot capacities = per-rank max) fits every core
  with ~11% padding. A pointer lives on the core that owns its graph, so there
  are no collectives and no data-dependent addressing.
* Per core: PE computes X^T[e, n] per graph (e on partitions) and
  y^T[e, slot] = Wy^T pred + (bx+by).  DVE computes t = relu(X^T + y_col)
  with a fused per-partition add+max.  PE contracts t with ws via an
  accumulating matmul whose stationary is a sliding window over a buffer
  holding ws at one column: pair r of slots lands in PSUM row r, giving a
  single [npairs, 512] result tile instead of 512 one-partition copies.
* Host does the cheap O(MB) reshapes: pre-transposing operands into
  [contraction-on-partition] layout, slot packing, final unpermute + -inf mask.
"""

import numpy as np
import ml_dtypes

N_GRAPHS, MAX_NODES, DIM, N_PTR = 64, 256, 1024, 512
N_CORES = 8
G_PER_CORE = N_GRAPHS // N_CORES  # 8
KT = DIM // 128  # 8 contraction tiles
NEG_INF = np.float32(-np.inf)

# matmul / relu operand dtype ("bfloat16" or "float32")
MM_DTYPE = "bfloat16"
# of every 12 relu tiles, how many go to ScalarE (rest on VectorE)
ACT_RELU_OF_12 = 4


def _np_dt(name):
    return ml_dtypes.bfloat16 if name == "bfloat16" else np.float32


def _build_program(npairs, slot_graph, n_slots):
    """One SPMD program; per-core behaviour differs only through data.

    slot_graph[s] = local graph index (0..7) whose X^T slot s reads.
    n_slots = 2 * npairs (slot count padded to even).
    """
    import concourse.bacc as bacc
    import concourse.tile as tile
    from concourse import mybir
    from concourse.bass import ts, ds

    mdt = getattr(mybir.dt, MM_DTYPE)
    f32 = mybir.dt.float32
    wwin = 2 * npairs - 1  # ws window buffer width

    nc = bacc.Bacc()
    embT_d = nc.declare_dram_parameter("embT", [KT, 128, G_PER_CORE, MAX_NODES], mdt, isOutput=False)
    predT_d = nc.declare_dram_parameter("predT", [KT, 128, n_slots], mdt, isOutput=False)
    WxT_d = nc.declare_dram_parameter("WxT", [KT, 128, DIM], mdt, isOutput=False)
    WyT_d = nc.declare_dram_parameter("WyT", [KT, 128, DIM], mdt, isOutput=False)
    bias_d = nc.declare_dram_parameter("biasv", [128, KT], f32, isOutput=False)
    wsbig_d = nc.declare_dram_parameter("wsbig", [KT, 128, wwin], mdt, isOutput=False)
    out_d = nc.declare_dram_parameter("scores_out", [npairs, 512], f32, isOutput=True)

    with tile.TileContext(nc) as tc:
        with (
            tc.tile_pool(name="const", bufs=1) as cpool,
            tc.tile_pool(name="t", bufs=10) as tpool,
            tc.tile_pool(name="psx", bufs=5, space="PSUM") as psx,
            tc.tile_pool(name="psy", bufs=2, space="PSUM") as psy,
            tc.tile_pool(name="pss", bufs=1, space="PSUM") as pss,
        ):
            embT_sb = cpool.tile([128, KT, G_PER_CORE, MAX_NODES], mdt)
            WxT_sb = cpool.tile([128, KT, DIM], mdt)
            WyT_sb = cpool.tile([128, KT, DIM], mdt)
            predT_sb = cpool.tile([128, KT, n_slots], mdt)
            wsbig_sb = cpool.tile([128, KT, wwin], mdt)
            bias_sb = cpool.tile([128, KT], f32)
            XT_sb = cpool.tile([128, KT, G_PER_CORE, MAX_NODES], mdt)
            yT_sb = cpool.tile([128, KT, n_slots], f32)
            scores_sb = cpool.tile([npairs, 512], f32)

            # emb/Wx first: X-proj(et=0) is the critical path at kernel start
            nc.sync.dma_start(bias_sb[:], bias_d[:])
            for dt in range(KT):
                nc.sync.dma_start(embT_sb[:, dt], embT_d[dt])
                nc.sync.dma_start(WxT_sb[:, dt], WxT_d[dt])
            for dt in range(KT):
                nc.sync.dma_start(predT_sb[:, dt], predT_d[dt])
                nc.sync.dma_start(WyT_sb[:, dt], WyT_d[dt])
                nc.sync.dma_start(wsbig_sb[:, dt], wsbig_d[dt])

            # y^T[e, s] = sum_d Wy[e, d] pred[s, d] + (bx + by)[e]
            for et in range(KT):
                py = psy.tile([128, n_slots], f32)
                for dt in range(KT):
                    nc.tensor.matmul(
                        py[:],
                        WyT_sb[:, dt, ts(et, 128)],
                        predT_sb[:, dt, :],
                        start=(dt == 0),
                        stop=(dt == KT - 1),
                    )
                nc.vector.tensor_scalar(
                    yT_sb[:, et, :], py[:], bias_sb[:, et : et + 1], None,
                    mybir.AluOpType.add,
                )

            # Interleaved per e-tile: X^T[e, n] = sum_d Wx[e, d] emb[n, d]
            # (two graphs per psum tile, dt-outer so 4 consecutive matmuls
            # share lhsT), immediately followed by the score contraction for
            # k-tile kt = et. PSUM accumulation is order-free, so emitting
            # the score chain kt-outer lets PE/DVE/ACT pipeline across
            # e-tiles instead of serializing X-proj before all scores.
            ps = pss.tile([npairs, 512], f32)
            n_mm = npairs * KT
            mm = 0

            def emit_score(kt, p2):
                nonlocal mm
                s0, s1 = 2 * p2, 2 * p2 + 1
                t = tpool.tile([128, 512], mdt, name="t", tag="t")
                for half, s in ((0, s0), (1, s1)):
                    # relu(X^T + y): balance between DVE and ACT
                    if (mm * 2 + half) % 12 < ACT_RELU_OF_12:
                        nc.scalar.activation(
                            t[:, ts(half, 256)],
                            XT_sb[:, kt, slot_graph[s], :],
                            mybir.ActivationFunctionType.Relu,
                            bias=yT_sb[:, kt, s : s + 1],
                            scale=1.0,
                        )
                    else:
                        nc.vector.tensor_scalar(
                            t[:, ts(half, 256)],
                            XT_sb[:, kt, slot_graph[s], :],
                            yT_sb[:, kt, s : s + 1],
                            0.0,
                            mybir.AluOpType.add,
                            mybir.AluOpType.max,
                        )
                nc.tensor.matmul(
                    ps[:],
                    wsbig_sb[:, kt, ds(npairs - 1 - p2, npairs)],
                    t[:],
                    start=(mm == 0),
                    stop=(mm == n_mm - 1),
                )
                mm += 1

            # One-iteration skew: scores for k-tile et-1 are emitted in chunks
            # between the dt-groups of X-proj(et), so PE always has X matmuls
            # queued while DVE/ACT produce relu tiles.
            chunk = (npairs + KT - 1) // KT  # score pairs per dt-group
            for et in range(KT + 1):
                skt = et - 1  # score k-tile trailing the X-proj e-tile
                if et < KT:
                    pxs = [psx.tile([128, 512], f32, name="px", tag="px") for _ in range(G_PER_CORE // 2)]
                    for dt in range(KT):
                        for l2 in range(G_PER_CORE // 2):
                            nc.tensor.matmul(
                                pxs[l2][:],
                                WxT_sb[:, dt, ts(et, 128)],
                                embT_sb[:, dt, 2 * l2 : 2 * l2 + 2, :],
                                start=(dt == 0),
                                stop=(dt == KT - 1),
                            )
                        if skt >= 0:
                            for p2 in range(dt * chunk, min((dt + 1) * chunk, npairs)):
                                emit_score(skt, p2)
                    for l2 in range(G_PER_CORE // 2):
                        nc.scalar.copy(XT_sb[:, et, 2 * l2 : 2 * l2 + 2, :], pxs[l2][:])
                else:
                    for p2 in range(npairs):
                        emit_score(skt, p2)

            nc.vector.tensor_copy(scores_sb[:], ps[:])
            nc.sync.dma_start(out_d[:], scores_sb[:])

    nc.finalize()
    return nc


def _prepare(graph_sizes, embeddings, predicted_embs, graph_ids, Wx, bx, Wy, by, ws):
    np_dt = _np_dt(MM_DTYPE)
    counts = np.bincount(graph_ids, minlength=N_GRAPHS)
    order = np.argsort(-counts, kind="stable")  # rank -> graph id

    S = [int(counts[order[G_PER_CORE * l]]) for l in range(G_PER_CORE)]
    T = sum(S)
    n_slots = T + (T % 2)
    npairs = n_slots // 2

    slot_graph = []
    for l in range(G_PER_CORE):
        slot_graph += [l] * S[l]
    if T % 2:
        slot_graph.append(slot_graph[-1] if slot_graph else 0)

    slot_start = np.cumsum([0] + S)

    WxT = np.ascontiguousarray(Wx.T).reshape(KT, 128, DIM).astype(np_dt)
    WyT = np.ascontiguousarray(Wy.T).reshape(KT, 128, DIM).astype(np_dt)
    biasv = np.ascontiguousarray((bx + by).astype(np.float32).reshape(KT, 128).T)
    wwin = 2 * npairs - 1
    wsbig = np.zeros((KT, 128, wwin), dtype=np_dt)
    wsbig[:, :, npairs - 1] = ws.reshape(KT, 128).astype(np_dt)

    in_maps = []
    ptr_of_slot = []  # per core: global pointer index per slot (-1 = pad)
    for j in range(N_CORES):
        graphs_j = order[[G_PER_CORE * l + j for l in range(G_PER_CORE)]]
        embT = np.ascontiguousarray(
            embeddings[graphs_j].transpose(2, 0, 1)
        ).reshape(KT, 128, G_PER_CORE, MAX_NODES).astype(np_dt)

        slots = np.full(n_slots, -1, dtype=np.int64)
        predc = np.zeros((DIM, n_slots), dtype=np.float32)
        for l in range(G_PER_CORE):
            ptrs = np.nonzero(graph_ids == graphs_j[l])[0]
            k = len(ptrs)
            slots[slot_start[l] : slot_start[l] + k] = ptrs
            predc[:, slot_start[l] : slot_start[l] + k] = predicted_embs[ptrs].T
        ptr_of_slot.append(slots)
        predT = np.ascontiguousarray(predc.reshape(KT, 128, n_slots)).astype(np_dt)

        in_maps.append({
            "embT": embT, "predT": predT, "WxT": WxT, "WyT": WyT,
            "biasv": biasv, "wsbig": wsbig,
        })

    return in_maps, ptr_of_slot, npairs, slot_graph, n_slots


def _run(inputs, trace=False):
    from concourse.bass_utils import run_bass_kernel_spmd

    graph_sizes = np.asarray(inputs["graph_sizes"])
    embeddings = np.asarray(inputs["embeddings"], dtype=np.float32)
    predicted_embs = np.asarray(inputs["predicted_embs"], dtype=np.float32)
    graph_ids = np.asarray(inputs["graph_ids"]).astype(np.int64)
    Wx = np.asarray(inputs["Wx"], dtype=np.float32)
    bx = np.asarray(inputs["bx"], dtype=np.float32)
    Wy = np.asarray(inputs["Wy"], dtype=np.float32)
    by = np.asarray(inputs["by"], dtype=np.float32)
    ws = np.asarray(inputs["ws"], dtype=np.float32)

    in_maps, ptr_of_slot, npairs, slot_graph, n_slots = _prepare(
        graph_sizes, embeddings, predicted_embs, graph_ids, Wx, bx, Wy, by, ws
    )
    nc = _build_program(npairs, slot_graph, n_slots)
    res = run_bass_kernel_spmd(nc, in_maps, list(range(N_CORES)), trace=trace)

    scores = np.zeros((N_PTR, MAX_NODES), dtype=np.float32)
    for j in range(N_CORES):
        rows = np.asarray(res.results[j]["scores_out"], dtype=np.float32)
        rows = rows.reshape(n_slots, MAX_NODES)
        valid = ptr_of_slot[j] >= 0
        scores[ptr_of_slot[j][valid]] = rows[valid]

    sizes = graph_sizes[graph_ids].astype(np.int64)
    pad = np.arange(MAX_NODES, dtype=np.int64)[None, :] >= sizes[:, None]
    scores = np.where(pad, NEG_INF, scores)
    return scores, res


def kernel(**inputs):
    scores, _ = _run(inputs, trace=False)
    return scores
